# revision 8
# baseline (speedup 1.0000x reference)
"""Trainium2 Bass kernel for nn_Coupling: out[e, s*J+j] = sum_a feat[e, a*S+s] * P[a, j].

Sharding: env axis data-parallel across 8 cores (3750 envs/core); P is tiny and
built host-side, replicated to every core as a [108, 30] block-diagonal input.

bf16 pipeline (error budget ~3e-3 « 2e-2 gate): host casts features/P to bf16,
device computes bf16 matmuls with fp32 PSUM accumulation, writes bf16 output,
host casts back to fp32.  Halves HBM traffic on both sides.

Per-core device kernel:
  - K-packs 3 envs into one contraction (K = 3*36 = 108 partitions) against the
    block-diagonal P ([108, 30] moving operand, 30 = 3 envs x 10 j).
  - A batch = up to 128 triples (384 envs).  Phase r (r = 0..255) does ONE
    matmul: stationary ft[:, :, r] ([108, n_tri_b] -> output partition = triple),
    moving P-block, output [n_tri_b, 30] into PSUM at phase slot r.  Every
    feature column crosses LDWEIGHTS exactly once (the PE-side floor).
  - PSUM phase slots are padded to 32 elements (128 B) so a 30-float matmul
    output never straddles a 2 KB PSUM bank; 64 phases per PSUM tile (4 banks).
  - One DVE copy per PSUM tile permutes (r,t,j) -> (t,r,j) bf16 into a stage
    buffer whose per-partition rows are that triple's 3 full env rows --
    so the flush is a single fully-contiguous ~2 MB DMA per batch.
"""

import numpy as np
import ml_dtypes

import concourse.mybir as mybir
from concourse import bacc, tile
from concourse.bass_utils import run_bass_kernel_spmd

ENV = 30000
A = 36          # n_alpha
S = 256         # soap
J = 10          # n_j
N_CORES = 8
E_CORE = ENV // N_CORES  # 3750

T = 3           # envs packed into one contraction (K = T*A = 108)
K = T * A
TRI_B = 128     # triples per batch (= output partitions per batch)
RB = 64         # phases per PSUM tile (64 * 32 * 4 B = 4 banks)

F32 = mybir.dt.float32
BF16 = mybir.dt.bfloat16

_NC_CACHE = {}


def build_nc(n_env, tri_b=TRI_B, fbufs=8, stbufs=2, psbufs=2):
    assert n_env % T == 0
    n_tri = n_env // T
    H = 32  # triples per feature sub-tile (= matmul column-group width)

    nc = bacc.Bacc("TRN2", target_bir_lowering=False, debug=False)

    feat = nc.dram_tensor("features", [n_env, A * S], BF16, kind="ExternalInput")
    pblk = nc.dram_tensor("pblk", [K, T * J], BF16, kind="ExternalInput")
    out = nc.dram_tensor("out", [n_env, S * J], BF16, kind="ExternalOutput")

    feat3 = feat.rearrange("e (a s) -> e a s", a=A)

    with tile.TileContext(nc) as tc:
        with (
            tc.tile_pool(name="const", bufs=1) as cpool,
            tc.tile_pool(name="feat", bufs=fbufs) as fpool,
            tc.tile_pool(name="psum", bufs=psbufs, space="PSUM") as pspool,
            tc.tile_pool(name="stage", bufs=stbufs) as stpool,
        ):
            pb = cpool.tile([K, T * J], BF16)
            nc.sync.dma_start(pb[:], pblk[:])

            for tb in range(0, n_tri, tri_b):
                ntb = min(tri_b, n_tri - tb)
                eb = tb * T

                # feature sub-tiles of <=32 triples: loads for batch k+1 can
                # dispatch as soon as batch k-1's matmuls release the buffer,
                # a full batch of prefetch slack
                fts = []
                for h0 in range(0, ntb, H):
                    nh = min(H, ntb - h0)
                    fth = fpool.tile([K, nh, S], BF16)
                    nc.sync.dma_start(
                        fth[:],
                        feat3[eb + h0 * T : eb + (h0 + nh) * T].rearrange(
                            "(m t) a s -> t a m s", t=T
                        ),
                    )
                    fts.append(fth)

                stage = stpool.tile([ntb, T, S * J], BF16)
                stage4 = stage.rearrange("p t (r j) -> p t r j", j=J)

                for blk in range(S // RB):
                    ps = pspool.tile([ntb, RB, 32], F32)
                    for rr in range(RB):
                        r = blk * RB + rr
                        for hi, fth in enumerate(fts):
                            h0 = hi * H
                            nh = fth.shape[1]
                            nc.tensor.matmul(
                                ps[h0 : h0 + nh, rr, 0:J * T],
                                fth[:, :, r],
                                pb[:],
                                tile_position=(0, h0),
                            )
                    copy_eng = nc.vector if blk % 2 == 0 else nc.scalar
                    copy_fn = (
                        copy_eng.tensor_copy
                        if copy_eng is nc.vector
                        else copy_eng.copy
                    )
                    copy_fn(
                        stage4[:, :, blk * RB : (blk + 1) * RB],
                        ps[:, :, 0:J * T].rearrange("p r (t j) -> p t r j", t=T),
                    )

                nc.scalar.dma_start(
                    out[eb : eb + ntb * T].rearrange("(m t) x -> m t x", t=T),
                    stage[:],
                )

    nc.compile()
    return nc


def _get_nc(n_env, **kw):
    key = (n_env, tuple(sorted(kw.items())))
    if key not in _NC_CACHE:
        _NC_CACHE[key] = build_nc(n_env, **kw)
    return _NC_CACHE[key]


def make_pblk(U, alpha1, alpha2, j1, j2):
    P = (U[alpha1][:, j1] * U[alpha2][:, j2]).astype(np.float32)  # [A, J]
    pblk = np.zeros((K, T * J), dtype=np.float32)
    for t in range(T):
        pblk[t * A : (t + 1) * A, t * J : (t + 1) * J] = P
    return pblk.astype(ml_dtypes.bfloat16)


def run_spmd(features, U, alpha1, alpha2, j1, j2, trace=False, **kw):
    features = np.asarray(features, dtype=np.float32).astype(ml_dtypes.bfloat16)
    pblk = make_pblk(
        np.asarray(U), np.asarray(alpha1), np.asarray(alpha2),
        np.asarray(j1), np.asarray(j2),
    )
    nc = _get_nc(E_CORE, **kw)
    in_maps = [
        {"features": features[c * E_CORE : (c + 1) * E_CORE], "pblk": pblk}
        for c in range(N_CORES)
    ]
    res = run_bass_kernel_spmd(nc, in_maps, list(range(N_CORES)), trace=trace)
    out = np.concatenate(
        [res.results[c]["out"] for c in range(N_CORES)], axis=0
    ).astype(np.float32)
    return out, res


def kernel(features, U, alpha1, alpha2, j1, j2):
    return run_spmd(features, U, alpha1, alpha2, j1, j2)[0]


# revision 9
# speedup vs baseline: 1.2394x; 1.2394x over previous
"""Trainium2 Bass kernel for nn_Coupling: out[e, s*J+j] = sum_a feat[e, a*S+s] * P[a, j].

Sharding: env axis data-parallel across 8 cores (3750 envs/core); P is tiny and
built host-side, replicated to every core as a [108, 30] block-diagonal input.

bf16 pipeline (error budget ~3e-3 « 2e-2 gate): host casts features/P to bf16,
device computes bf16 matmuls with fp32 PSUM accumulation, writes bf16 output,
host casts back to fp32.  Halves HBM traffic on both sides.

Per-core device kernel:
  - K-packs 3 envs into one contraction (K = 3*36 = 108 partitions) against the
    block-diagonal P ([108, 30] moving operand, 30 = 3 envs x 10 j).
  - A batch = up to 128 triples (384 envs).  Phase r (r = 0..255) does ONE
    matmul: stationary ft[:, :, r] ([108, n_tri_b] -> output partition = triple),
    moving P-block, output [n_tri_b, 30] into PSUM at phase slot r.  Every
    feature column crosses LDWEIGHTS exactly once (the PE-side floor).
  - PSUM phase slots are padded to 32 elements (128 B) so a 30-float matmul
    output never straddles a 2 KB PSUM bank; 64 phases per PSUM tile (4 banks).
  - One DVE copy per PSUM tile permutes (r,t,j) -> (t,r,j) bf16 into a stage
    buffer whose per-partition rows are that triple's 3 full env rows --
    so the flush is a single fully-contiguous ~2 MB DMA per batch.
"""

import numpy as np
import ml_dtypes

import concourse.mybir as mybir
from concourse import bacc, tile
from concourse.bass_utils import run_bass_kernel_spmd

ENV = 30000
A = 36          # n_alpha
S = 256         # soap
J = 10          # n_j
N_CORES = 8
E_CORE = ENV // N_CORES  # 3750

T = 3           # envs packed into one contraction (K = T*A = 108)
K = T * A
TRI_B = 128     # triples per batch (= output partitions per batch)
RB = 64         # phases per PSUM tile (64 * 32 * 4 B = 4 banks)

F32 = mybir.dt.float32
BF16 = mybir.dt.bfloat16

_NC_CACHE = {}


def build_nc(n_env, tri_b=TRI_B, fbufs=5, stbufs=2, psbufs=2, H=64):
    assert n_env % T == 0
    n_tri = n_env // T

    nc = bacc.Bacc("TRN2", target_bir_lowering=False, debug=False)

    feat = nc.dram_tensor("features", [n_env, A * S], BF16, kind="ExternalInput")
    pblk = nc.dram_tensor("pblk", [K, T * J], BF16, kind="ExternalInput")
    out = nc.dram_tensor("out", [n_env, S * J], BF16, kind="ExternalOutput")

    feat3 = feat.rearrange("e (a s) -> e a s", a=A)

    with tile.TileContext(nc) as tc:
        with (
            tc.tile_pool(name="const", bufs=1) as cpool,
            tc.tile_pool(name="feat", bufs=fbufs) as fpool,
            tc.tile_pool(name="psum", bufs=psbufs, space="PSUM") as pspool,
            tc.tile_pool(name="stage", bufs=stbufs) as stpool,
        ):
            pb = cpool.tile([K, T * J], BF16)
            nc.sync.dma_start(pb[:], pblk[:])

            for tb in range(0, n_tri, tri_b):
                ntb = min(tri_b, n_tri - tb)
                eb = tb * T

                # feature sub-tiles of <=32 triples: loads for batch k+1 can
                # dispatch as soon as batch k-1's matmuls release the buffer,
                # a full batch of prefetch slack
                fts = []
                for h0 in range(0, ntb, H):
                    nh = min(H, ntb - h0)
                    fth = fpool.tile([K, nh, S], BF16)
                    nc.sync.dma_start(
                        fth[:],
                        feat3[eb + h0 * T : eb + (h0 + nh) * T].rearrange(
                            "(m t) a s -> t a m s", t=T
                        ),
                    )
                    fts.append(fth)

                stage = stpool.tile([ntb, T, S * J], BF16)
                stage4 = stage.rearrange("p t (r j) -> p t r j", j=J)

                for blk in range(S // RB):
                    ps = pspool.tile([ntb, RB, 32], F32)
                    for rr in range(RB):
                        r = blk * RB + rr
                        for hi, fth in enumerate(fts):
                            h0 = hi * H
                            nh = fth.shape[1]
                            nc.tensor.matmul(
                                ps[h0 : h0 + nh, rr, 0:J * T],
                                fth[:, :, r],
                                pb[:],
                                tile_position=(0, h0),
                            )
                    copy_eng = nc.vector if blk % 2 == 0 else nc.scalar
                    copy_fn = (
                        copy_eng.tensor_copy
                        if copy_eng is nc.vector
                        else copy_eng.copy
                    )
                    copy_fn(
                        stage4[:, :, blk * RB : (blk + 1) * RB],
                        ps[:, :, 0:J * T].rearrange("p r (t j) -> p t r j", t=T),
                    )

                nc.scalar.dma_start(
                    out[eb : eb + ntb * T].rearrange("(m t) x -> m t x", t=T),
                    stage[:],
                )

    nc.compile()
    return nc


def _get_nc(n_env, **kw):
    key = (n_env, tuple(sorted(kw.items())))
    if key not in _NC_CACHE:
        _NC_CACHE[key] = build_nc(n_env, **kw)
    return _NC_CACHE[key]


def make_pblk(U, alpha1, alpha2, j1, j2):
    P = (U[alpha1][:, j1] * U[alpha2][:, j2]).astype(np.float32)  # [A, J]
    pblk = np.zeros((K, T * J), dtype=np.float32)
    for t in range(T):
        pblk[t * A : (t + 1) * A, t * J : (t + 1) * J] = P
    return pblk.astype(ml_dtypes.bfloat16)


def run_spmd(features, U, alpha1, alpha2, j1, j2, trace=False, **kw):
    features = np.asarray(features, dtype=np.float32).astype(ml_dtypes.bfloat16)
    pblk = make_pblk(
        np.asarray(U), np.asarray(alpha1), np.asarray(alpha2),
        np.asarray(j1), np.asarray(j2),
    )
    nc = _get_nc(E_CORE, **kw)
    in_maps = [
        {"features": features[c * E_CORE : (c + 1) * E_CORE], "pblk": pblk}
        for c in range(N_CORES)
    ]
    res = run_bass_kernel_spmd(nc, in_maps, list(range(N_CORES)), trace=trace)
    out = np.concatenate(
        [res.results[c]["out"] for c in range(N_CORES)], axis=0
    ).astype(np.float32)
    return out, res


def kernel(features, U, alpha1, alpha2, j1, j2):
    return run_spmd(features, U, alpha1, alpha2, j1, j2)[0]
